# revision 1
# baseline (speedup 1.0000x reference)
"""MoE (top-K routing, per-expert capacity) Trainium2 kernel.

Strategy: expert parallelism across 8 NeuronCores (E=8, one expert per core).
 - Host: routing top-C selection per expert (tiny: E x T scores), gather of
   dispatched tokens, and fold of the combine weights ("gain") into the
   dispatched activations. gain >= 0 (softmax outputs), so
   gain * (relu(xe@W1)@W2) == relu((gain*xe)@W1)@W2 exactly in math terms.
 - Device (per core): fused 2-layer MLP in a single hand-written Tile kernel:
       hT = relu(W1.T @ xeT)   (F, Ca)   hT kept in SBUF, F in G groups
       y  = hT.T @ W2          (Ca, D)   PSUM-accumulated per group,
                                         DVE-accumulated across groups
   float32r matmuls (full PE stream rate, fp32-class data).
 - Host: per-expert scatter-add of y_e back into the (T, D) output.

Only the active capacity prefix Ca <= C is computed: top-C ordering sorts
valid slots first, so slots >= max_e(n_routed_e) are structurally zero.
Programs are cached per Ca (multiple of 128), so any input works.

W1 is host-packed into (F/128, 128, D/128, 128) blocks so each stationary
tile streams as contiguous 4KB runs per partition (512B runs measured at
~50GB/s vs ~contiguous at full rate).

b1/b2 are structurally zero in this problem (setup_inputs fills zeros); a
host-side fallback handles nonzero b2, and nonzero b1 is unsupported.
"""

import math
import sys

import numpy as np

for _p in ("/opt/trn_rl_repo",):
    if _p not in sys.path:
        sys.path.append(_p)

# Problem dims (hardcoded per contract)
T, E, D, F, C, K = 4096, 8, 1024, 4096, 1536, 2
N_CORES = 8
P = 128
G = 4  # F-dim groups for the fused hT staging
KO = D // P  # 8 k-subtiles of the D contraction
NF = F // P  # 32 f-chunks of 128
FPG = NF // G  # f-chunks per group

_PROGRAMS = {}  # c_act -> (nc, names)


def _c_chunks(c_act):
    """Split c_act into matmul free-dim chunks <= 512, preferring >= 256
    (float32r streams at 1 cyc/row only for N >= 256)."""
    chunks = []
    rem = c_act
    while rem > 0:
        if rem > 512:
            if rem - 512 >= 256 or rem == 1024:
                take = 512
            else:  # rem in (512, 768): split evenly-ish to keep both >= 256
                take = 384
        else:
            take = rem
        chunks.append(take)
        rem -= take
    return chunks


def _build_program(c_act):
    import concourse.mybir as mybir
    import concourse.tile as tile
    from concourse import bacc

    f32 = mybir.dt.float32
    f32r = mybir.dt.float32r
    Relu = mybir.ActivationFunctionType.Relu

    CS = c_act // P  # c-subtiles for MM2
    ND = D // 512  # 2 n-chunks of 512 for MM2
    chunks = _c_chunks(c_act)

    nc = bacc.Bacc(None, target_bir_lowering=False, debug=False)

    with tile.TileContext(nc) as tc:
        with tc.tile_pool(name="dram", bufs=1, space="DRAM") as dram:
            # w1 block-packed on host: (NF, P, KO, P); [fg] -> [ki, ko, f] tile
            w1 = dram.tile((NF, P, KO, P), f32r, kind="ExternalInput", name="w1")
            w2 = dram.tile((F, D), f32r, kind="ExternalInput", name="w2")
            xeT = dram.tile((D, c_act), f32r, kind="ExternalInput", name="xeT")
            y = dram.tile((c_act, D), f32, kind="ExternalOutput", name="y")

        xeT_r = xeT[:].rearrange("(ko ki) c -> ki ko c", ki=P)

        with (
            tc.tile_pool(name="const", bufs=1) as constp,
            tc.tile_pool(name="xe", bufs=1) as xep,
            tc.tile_pool(name="ht", bufs=1) as htp,
            tc.tile_pool(name="ysb", bufs=1) as yp,
            tc.tile_pool(name="w1t", bufs=6) as w1p,
            # SBUF/partition ~= 96*c_act B + w2 bufs*32KB + ~45KB; cap 208KB
            tc.tile_pool(name="w2t", bufs=2 if c_act <= 1216 else 1) as w2p,
            tc.tile_pool(name="ps", bufs=2, space="PSUM") as psp,
        ):
            zero = constp.tile([P, 1], f32)
            nc.any.memset(zero[:], 0.0)

            # HAM warm-up: ~6us of dependency-free matmuls on a memset tile
            # fill the PE activity window while xe/w1 stream in, so the real
            # matmuls start at 2.4GHz instead of the cold 1.2GHz
            bf16 = mybir.dt.bfloat16
            warm_w = constp.tile([P, P], bf16)
            nc.any.memset(warm_w[:], 0.0)
            warm_sb = constp.tile([P, 512], bf16)
            nc.any.memset(warm_sb[:], 0.0)
            warm_out = constp.tile([P, 1], f32)
            with tc.tile_pool(name="warmps", bufs=1, space="PSUM") as warmp:
                warm_ps = warmp.tile([P, 512], f32)
                N_WARM = 16
                for i in range(N_WARM):
                    nc.tensor.matmul(
                        warm_ps[:], warm_w[:], warm_sb[:], start=True, stop=True
                    )
                nc.vector.tensor_copy(warm_out[:], warm_ps[:, :1])

            # first stationary tile ahead of everything: it heads its DMA
            # queue so the PE can start ~10us in instead of ~25us
            w1_first = w1p.tile([P, KO, P], f32r, name="w1_t")
            nc.sync.dma_start(w1_first[:], w1[0])

            # xe split per (ko, chunk) in consumption order: small transfers
            # land progressively so MM1(fc0) streams as they arrive
            xe_sb = xep.tile([P, KO, c_act], f32r)
            for ko in range(KO):
                nc.sync.dma_start(xe_sb[:, ko, :], xeT_r[:, ko, :])

            y_sb = yp.tile([P, CS, D], f32)
            hT = htp.tile([P, FPG, c_act], f32r)

            # chunk index -> (c offset, width)
            offs = []
            c0 = 0
            for cw in chunks:
                offs.append((c0, cw))
                c0 += cw

            def mm1_sweep(g, idxs, use_first):
                """One fc-sweep of MM1 over the given c-chunk indices."""
                for fc in range(FPG):
                    fg = g * FPG + fc
                    if use_first and fc == 0:
                        w1_t = w1_first
                    else:
                        w1_t = w1p.tile([P, KO, P], f32r, name="w1_t")
                        nc.sync.dma_start(w1_t[:], w1[fg])
                    ph = {
                        i: psp.tile([P, chunks[i]], f32, name=f"p{i}", tag=f"p{i}")
                        for i in idxs
                    }
                    for k in range(KO):
                        # smallest chunk first: the trailing wide matmul
                        # hides the next k-step's LDWEIGHTS
                        for i in sorted(idxs, key=lambda j: chunks[j]):
                            c0, cw = offs[i]
                            nc.tensor.matmul(
                                ph[i][:],
                                w1_t[:, k, :],
                                xe_sb[:, k, c0 : c0 + cw],
                                start=(k == 0),
                                stop=(k == KO - 1),
                            )
                    for i in idxs:
                        c0, cw = offs[i]
                        nc.scalar.activation(
                            hT[:, fc, c0 : c0 + cw], ph[i][:], Relu, bias=zero[:]
                        )

            for g in range(G):
                # ---- MM1: hT[group] = relu(W1[:, group].T @ xeT) ----
                mm1_sweep(g, list(range(len(chunks))), use_first=(g == 0))

                # W2 tiles for this group (emitted after MM1 so the per-queue
                # DMA FIFOs serve the w1/xe tiles PE needs first)
                w2_t = w2p.tile([P, FPG, D], f32r, name="w2_t")
                for fs in range(FPG):
                    fg = g * FPG + fs
                    nc.sync.dma_start(w2_t[:, fs, :], w2[fg * P : (fg + 1) * P, :])

                # ---- MM2: y[group contribution] = hT.T @ W2[group] ----
                for cs in range(CS):
                    py = [
                        psp.tile([P, 512], f32, name=f"py{dh}", tag=f"p{dh}")
                        for dh in range(ND)
                    ]
                    for fs in range(FPG):
                        for dh in range(ND):
                            nc.tensor.matmul(
                                py[dh][:],
                                hT[:, fs, cs * P : (cs + 1) * P],
                                w2_t[:, fs, dh * 512 : (dh + 1) * 512],
                                start=(fs == 0),
                                stop=(fs == FPG - 1),
                            )
                    for dh in range(ND):
                        dst = y_sb[:, cs, dh * 512 : (dh + 1) * 512]
                        if g == 0:
                            nc.vector.tensor_copy(dst, py[dh][:])
                        else:
                            nc.vector.tensor_add(dst, dst, py[dh][:])
                        if g == G - 1:
                            # final c-subtile: 2 stores per half across queues
                            # so the only non-overlapped store tail is ~128KB
                            nsp = 2 if cs == CS - 1 else 1
                            for sp in range(nsp):
                                w = 512 // nsp
                                c0s = dh * 512 + sp * w
                                nc.sync.dma_start(
                                    y[cs * P : (cs + 1) * P, c0s : c0s + w],
                                    y_sb[:, cs, c0s : c0s + w],
                                )

    nc.compile()
    names = dict(w1=w1.name, w2=w2.name, xeT=xeT.name, y=y.name)
    return nc, names


def _get_program(c_act):
    if c_act not in _PROGRAMS:
        _PROGRAMS[c_act] = _build_program(c_act)
    return _PROGRAMS[c_act]


# test.py can set RUN_KWARGS (e.g. dict(trace=True)) and read LAST_RESULTS
RUN_KWARGS = {}
LAST_RESULTS = None


def kernel(x, route_mask, route_weight, W1, b1, W2, b2):
    from concourse.bass_utils import run_bass_kernel_spmd

    global LAST_RESULTS

    x = np.asarray(x, dtype=np.float32)
    route_mask = np.asarray(route_mask, dtype=bool)
    route_weight = np.asarray(route_weight, dtype=np.float32)
    W1 = np.asarray(W1, dtype=np.float32)
    W2 = np.asarray(W2, dtype=np.float32)
    b1 = np.asarray(b1, dtype=np.float32)
    b2 = np.asarray(b2, dtype=np.float32)
    if np.any(b1):
        raise NotImplementedError("nonzero b1 not supported")

    # --- routing: per-expert top-C tokens by route weight (ties -> lower idx) ---
    w_et = np.where(route_mask.T, route_weight.T, -np.inf)  # (E, T)
    order = np.argsort(-w_et, axis=1, kind="stable")[:, :C]  # (E, C) token ids
    vals = np.take_along_axis(w_et, order, axis=1)  # (E, C)
    valid = np.isfinite(vals)  # (E, C)
    gain = np.where(valid, vals, 0.0).astype(np.float32)  # (E, C)

    # active capacity: valid slots are a prefix (sorted by weight desc)
    n_e = valid.sum(axis=1)
    c_act = min(C, int(math.ceil(max(1, n_e.max()) / P)) * P)

    nc, names = _get_program(c_act)

    # --- dispatch: gather + fold gain, per expert ---
    in_maps = []
    for e in range(E):
        xe = x[order[e, :c_act]] * gain[e, :c_act][:, None]  # (Ca, D)
        xeT_np = np.ascontiguousarray(xe.T)  # (D, Ca)
        w1b = np.ascontiguousarray(
            W1[e].reshape(KO, P, NF, P).transpose(2, 1, 0, 3)
        )  # (NF, P, KO, P)
        in_maps.append({names["w1"]: w1b, names["xeT"]: xeT_np, names["w2"]: W2[e]})

    res = run_bass_kernel_spmd(nc, in_maps, list(range(N_CORES)), **RUN_KWARGS)
    LAST_RESULTS = res

    # --- combine: scatter-add per-expert outputs ---
    y = np.zeros((T, D), np.float32)
    for e in range(E):
        ye = res.results[e][names["y"]]  # (Ca, D)
        m = valid[e, :c_act]
        if np.any(b2):
            ye = ye + gain[e, :c_act][:, None] * b2[e][None, :]
        y[order[e, :c_act][m]] += ye[m]
    return y



# revision 2
# speedup vs baseline: 1.0922x; 1.0922x over previous
"""MoE (top-K routing, per-expert capacity) Trainium2 kernel.

Strategy: expert parallelism across 8 NeuronCores (E=8, one expert per core).
 - Host: routing top-C selection per expert (tiny: E x T scores), gather of
   dispatched tokens, and fold of the combine weights ("gain") into the
   dispatched activations. gain >= 0 (softmax outputs), so
   gain * (relu(xe@W1)@W2) == relu((gain*xe)@W1)@W2 exactly in math terms.
 - Device (per core): fused 2-layer MLP in a single hand-written Tile kernel:
       hT = relu(W1.T @ xeT)   (F, Ca)   hT kept in SBUF (bf16: fits whole)
       y  = hT.T @ W2          (Ca, D)   PSUM-accumulated over all 32
                                         f-subtiles in one sweep
 - Host: per-expert scatter-add of y_e back into the (T, D) output.

All matmul operands are bf16: measured on this HW a bf16 matmul streams
512 cols in 216ns vs fp32r's 227ns (fp32r pays an ~11ns/matmul stationary
4-byte-load tax), bf16 has no fp32r >=256-col restriction (so the active
capacity needs no 128-padding), and all DMA/SBUF traffic halves. End-to-end
bf16 error on this problem is ~3.7e-3 (tolerance 2e-2): bf16's 8 mantissa
bits give ~1.1e-3 rms per quantized operand.

Only the active capacity prefix Ca <= C is computed: top-C ordering sorts
valid slots first, so slots >= max_e(n_routed_e) are structurally zero.
Programs are cached per Ca (multiple of 16), so any input works.

W1 is host-packed into (F/128, 128, D/128, 128) blocks so each stationary
tile streams as contiguous 2KB runs per partition.

b1/b2 are structurally zero in this problem (setup_inputs fills zeros); a
host-side fallback handles nonzero b2, and nonzero b1 is unsupported.
"""

import math
import sys

import numpy as np

for _p in ("/opt/trn_rl_repo",):
    if _p not in sys.path:
        sys.path.append(_p)

# Problem dims (hardcoded per contract)
T, E, D, F, C, K = 4096, 8, 1024, 4096, 1536, 2
N_CORES = 8
P = 128
KO = D // P  # 8 k-subtiles of the D contraction
NF = F // P  # 32 f-chunks of 128
ND = D // 512  # 2 n-chunks of 512 for MM2

_PROGRAMS = {}  # c_act -> (nc, names)


def _c_chunks(c_act):
    """Split c_act into matmul free-dim chunks <= 512 (PSUM bank width)."""
    chunks = []
    rem = c_act
    while rem > 0:
        take = min(512, rem)
        chunks.append(take)
        rem -= take
    return chunks


def _build_program(c_act):
    import concourse.mybir as mybir
    import concourse.tile as tile
    from concourse import bacc

    f32 = mybir.dt.float32
    bf16 = mybir.dt.bfloat16
    Relu = mybir.ActivationFunctionType.Relu

    CS = (c_act + P - 1) // P  # c-subtiles for MM2
    last_cw = c_act - P * (CS - 1)
    chunks = _c_chunks(c_act)

    nc = bacc.Bacc(None, target_bir_lowering=False, debug=False)

    with tile.TileContext(nc) as tc:
        with tc.tile_pool(name="dram", bufs=1, space="DRAM") as dram:
            # w1 block-packed on host: (NF, P, KO, P); [fg] -> [ki, ko, f] tile
            w1 = dram.tile((NF, P, KO, P), bf16, kind="ExternalInput", name="w1")
            w2 = dram.tile((F, D), bf16, kind="ExternalInput", name="w2")
            xeT = dram.tile((D, c_act), bf16, kind="ExternalInput", name="xeT")
            y = dram.tile((c_act, D), bf16, kind="ExternalOutput", name="y")

        xeT_r = xeT[:].rearrange("(ko ki) c -> ki ko c", ki=P)

        with (
            tc.tile_pool(name="const", bufs=1) as constp,
            tc.tile_pool(name="xe", bufs=1) as xep,
            tc.tile_pool(name="ht", bufs=1) as htp,
            tc.tile_pool(name="w2sb", bufs=1) as w2p,
            tc.tile_pool(name="ysb", bufs=1) as yp,
            tc.tile_pool(name="w1t", bufs=6) as w1p,
            tc.tile_pool(name="ps", bufs=2, space="PSUM") as psp,
        ):
            zero = constp.tile([P, 1], f32)
            nc.any.memset(zero[:], 0.0)

            # HAM warm-up: ~3.5us of dependency-free matmuls on a memset tile
            # fill the PE activity window while xe/w1 stream in, so the real
            # matmuls start at 2.4GHz instead of the cold 1.2GHz
            warm_w = constp.tile([P, P], bf16)
            nc.any.memset(warm_w[:], 0.0)
            warm_sb = constp.tile([P, 512], bf16)
            nc.any.memset(warm_sb[:], 0.0)
            warm_out = constp.tile([P, 1], f32)
            with tc.tile_pool(name="warmps", bufs=1, space="PSUM") as warmp:
                warm_ps = warmp.tile([P, 512], f32)
                N_WARM = 16
                for i in range(N_WARM):
                    nc.tensor.matmul(
                        warm_ps[:], warm_w[:], warm_sb[:], start=True, stop=True
                    )
                nc.vector.tensor_copy(warm_out[:], warm_ps[:, :1])

            # first stationary tile ahead of everything: it heads its DMA
            # queue so the PE can start early
            w1_first = w1p.tile([P, KO, P], bf16, name="w1_t")
            nc.sync.dma_start(w1_first[:], w1[0])

            # xe per ko in consumption order so MM1(fc0) streams as they land
            xe_sb = xep.tile([P, KO, c_act], bf16)
            for ko in range(KO):
                nc.sync.dma_start(xe_sb[:, ko, :], xeT_r[:, ko, :])

            hT = htp.tile([P, NF, c_act], bf16)
            w2_sb = w2p.tile([P, NF, D], bf16)
            y_sb = yp.tile([P, CS, D], bf16)

            # chunk index -> (c offset, width)
            offs = []
            c0 = 0
            for cw in chunks:
                offs.append((c0, cw))
                c0 += cw
            # smallest chunk first: the trailing wide matmul hides the next
            # k-step's LDWEIGHTS
            order = sorted(range(len(chunks)), key=lambda j: chunks[j])

            # ---- MM1: hT = relu(W1.T @ xeT), f-chunk at a time ----
            for fc in range(NF):
                if fc == 0:
                    w1_t = w1_first
                else:
                    w1_t = w1p.tile([P, KO, P], bf16, name="w1_t")
                    nc.sync.dma_start(w1_t[:], w1[fc])
                # pace the W2 prefetch behind the w1 tiles in queue order;
                # all of W2 is resident in SBUF before MM2 begins
                nc.sync.dma_start(w2_sb[:, fc, :], w2[fc * P : (fc + 1) * P, :])
                ph = {
                    i: psp.tile([P, chunks[i]], f32, name=f"p{i}", tag=f"p{i}")
                    for i in range(len(chunks))
                }
                for k in range(KO):
                    for i in order:
                        c0, cw = offs[i]
                        nc.tensor.matmul(
                            ph[i][:],
                            w1_t[:, k, :],
                            xe_sb[:, k, c0 : c0 + cw],
                            start=(k == 0),
                            stop=(k == KO - 1),
                        )
                for i in range(len(chunks)):
                    c0, cw = offs[i]
                    nc.scalar.activation(
                        hT[:, fc, c0 : c0 + cw], ph[i][:], Relu, bias=zero[:]
                    )

            # ---- MM2: y = hT.T @ W2, single full-F sweep per c-subtile ----
            for cs in range(CS):
                cw = P if cs < CS - 1 else last_cw
                py = [
                    psp.tile([P, 512], f32, name=f"py{dh}", tag=f"p{dh}")
                    for dh in range(ND)
                ]
                for fs in range(NF):
                    for dh in range(ND):
                        nc.tensor.matmul(
                            py[dh][:cw, :],
                            hT[:, fs, cs * P : cs * P + cw],
                            w2_sb[:, fs, dh * 512 : (dh + 1) * 512],
                            start=(fs == 0),
                            stop=(fs == NF - 1),
                        )
                for dh in range(ND):
                    dst = y_sb[:cw, cs, dh * 512 : (dh + 1) * 512]
                    nc.vector.tensor_copy(dst, py[dh][:cw, :])
                    # spread stores; split the last subtiles finer so the
                    # non-overlapped store tail stays tiny
                    nsp = 2 if cs >= CS - 2 else 1
                    for sp in range(nsp):
                        w = 512 // nsp
                        c0s = dh * 512 + sp * w
                        nc.sync.dma_start(
                            y[cs * P : cs * P + cw, c0s : c0s + w],
                            y_sb[:cw, cs, c0s : c0s + w],
                        )

    nc.compile()
    names = dict(w1=w1.name, w2=w2.name, xeT=xeT.name, y=y.name)
    return nc, names


def _get_program(c_act):
    if c_act not in _PROGRAMS:
        _PROGRAMS[c_act] = _build_program(c_act)
    return _PROGRAMS[c_act]


# test.py can set RUN_KWARGS (e.g. dict(trace=True)) and read LAST_RESULTS
RUN_KWARGS = {}
LAST_RESULTS = None


def kernel(x, route_mask, route_weight, W1, b1, W2, b2):
    import ml_dtypes

    from concourse.bass_utils import run_bass_kernel_spmd

    global LAST_RESULTS

    bf = ml_dtypes.bfloat16

    x = np.asarray(x, dtype=np.float32)
    route_mask = np.asarray(route_mask, dtype=bool)
    route_weight = np.asarray(route_weight, dtype=np.float32)
    W1 = np.asarray(W1, dtype=np.float32)
    W2 = np.asarray(W2, dtype=np.float32)
    b1 = np.asarray(b1, dtype=np.float32)
    b2 = np.asarray(b2, dtype=np.float32)
    if np.any(b1):
        raise NotImplementedError("nonzero b1 not supported")

    # --- routing: per-expert top-C tokens by route weight (ties -> lower idx) ---
    w_et = np.where(route_mask.T, route_weight.T, -np.inf)  # (E, T)
    order = np.argsort(-w_et, axis=1, kind="stable")[:, :C]  # (E, C) token ids
    vals = np.take_along_axis(w_et, order, axis=1)  # (E, C)
    valid = np.isfinite(vals)  # (E, C)
    gain = np.where(valid, vals, 0.0).astype(np.float32)  # (E, C)

    # active capacity: valid slots are a prefix (sorted by weight desc)
    n_e = valid.sum(axis=1)
    c_act = min(C, int(math.ceil(max(1, n_e.max()) / 16)) * 16)

    nc, names = _get_program(c_act)

    # --- dispatch: gather + fold gain, per expert ---
    in_maps = []
    for e in range(E):
        xe = x[order[e, :c_act]] * gain[e, :c_act][:, None]  # (Ca, D)
        xeT_np = np.ascontiguousarray(xe.T.astype(bf))  # (D, Ca)
        w1b = np.ascontiguousarray(
            W1[e].reshape(KO, P, NF, P).transpose(2, 1, 0, 3).astype(bf)
        )  # (NF, P, KO, P)
        in_maps.append(
            {names["w1"]: w1b, names["xeT"]: xeT_np, names["w2"]: W2[e].astype(bf)}
        )

    res = run_bass_kernel_spmd(nc, in_maps, list(range(N_CORES)), **RUN_KWARGS)
    LAST_RESULTS = res

    # --- combine: scatter-add per-expert outputs ---
    y = np.zeros((T, D), np.float32)
    for e in range(E):
        ye = res.results[e][names["y"]].astype(np.float32)  # (Ca, D)
        m = valid[e, :c_act]
        if np.any(b2):
            ye = ye + gain[e, :c_act][:, None] * b2[e][None, :]
        y[order[e, :c_act][m]] += ye[m]
    return y


# revision 10
# speedup vs baseline: 1.1030x; 1.0100x over previous
"""MoE (top-K routing, per-expert capacity) Trainium2 kernel.

Strategy: expert parallelism across 8 NeuronCores (E=8, one expert per core).
 - Host: routing top-C selection per expert (tiny: E x T scores), gather of
   dispatched tokens, and fold of the combine weights ("gain") into the
   dispatched activations. gain >= 0 (softmax outputs), so
   gain * (relu(xe@W1)@W2) == relu((gain*xe)@W1)@W2 exactly in math terms.
 - Device (per core): fused 2-layer MLP in a single hand-written Tile kernel:
       hT = relu(W1.T @ xeT)   (F, Ca)   hT kept in SBUF (bf16: fits whole)
       yT = W2.T @ hT          (D, Ca)   W2 128x128 chunks stationary, hT
                                         moving: PE cost scales with the
                                         exact Ca (no 128-col rounding),
                                         PSUM-accumulated over all 32
                                         f-subtiles in one sweep
 - Host: transpose + per-expert scatter-add of y_e into the (T, D) output.

All matmul operands are bf16: measured on this HW a bf16 matmul streams
512 cols in 216ns vs fp32r's 227ns (fp32r pays an ~11ns/matmul stationary
4-byte-load tax), bf16 has no fp32r >=256-col restriction (so the active
capacity needs no 128-padding), and all DMA/SBUF traffic halves. End-to-end
bf16 error on this problem is ~3.7e-3 (tolerance 2e-2): bf16's 8 mantissa
bits give ~1.1e-3 rms per quantized operand.

Only the active capacity prefix Ca <= C is computed: top-C ordering sorts
valid slots first, so slots >= max_e(n_routed_e) are structurally zero.
Programs are cached per Ca (multiple of 16), so any input works.

W1 is host-packed into (F/128, 128, D/128, 128) blocks so each stationary
tile streams as contiguous 2KB runs per partition.

b1/b2 are structurally zero in this problem (setup_inputs fills zeros); a
host-side fallback handles nonzero b2, and nonzero b1 is unsupported.
"""

import math
import sys

import numpy as np

for _p in ("/opt/trn_rl_repo",):
    if _p not in sys.path:
        sys.path.append(_p)

# Problem dims (hardcoded per contract)
T, E, D, F, C, K = 4096, 8, 1024, 4096, 1536, 2
N_CORES = 8
P = 128
KO = D // P  # 8 k-subtiles of the D contraction
NF = F // P  # 32 f-chunks of 128
ND = D // 512  # 2 n-chunks of 512 for MM2

_PROGRAMS = {}  # c_act -> (nc, names)


def _c_chunks(c_act):
    """Split c_act into matmul free-dim chunks <= 512 (PSUM bank width)."""
    chunks = []
    rem = c_act
    while rem > 0:
        take = min(512, rem)
        chunks.append(take)
        rem -= take
    return chunks


def _build_program(c_act):
    import concourse.mybir as mybir
    import concourse.tile as tile
    from concourse import bacc

    f32 = mybir.dt.float32
    bf16 = mybir.dt.bfloat16
    Relu = mybir.ActivationFunctionType.Relu
    Ident = mybir.ActivationFunctionType.Identity

    NDC = D // P  # 8 d-chunks for MM2 stationary
    chunks = _c_chunks(c_act)

    nc = bacc.Bacc(None, target_bir_lowering=False, debug=False)

    with tile.TileContext(nc) as tc:
        with tc.tile_pool(name="dram", bufs=1, space="DRAM") as dram:
            # w1 block-packed on host: (NF, P, KO, P); [fg] -> [ki, ko, f] tile
            w1 = dram.tile((NF, P, KO, P), bf16, kind="ExternalInput", name="w1")
            w2 = dram.tile((F, D), bf16, kind="ExternalInput", name="w2")
            xeT = dram.tile((D, c_act), bf16, kind="ExternalInput", name="xeT")
            yT = dram.tile((D, c_act), bf16, kind="ExternalOutput", name="yT")

        xeT_r = xeT[:].rearrange("(ko ki) c -> ki ko c", ki=P)

        with (
            tc.tile_pool(name="const", bufs=1) as constp,
            tc.tile_pool(name="xe", bufs=1) as xep,
            tc.tile_pool(name="ht", bufs=1) as htp,
            tc.tile_pool(name="w2sb", bufs=1) as w2p,
            tc.tile_pool(name="ysb", bufs=1) as yp,
            tc.tile_pool(name="w1t", bufs=6) as w1p,
            tc.tile_pool(name="ps", bufs=2, space="PSUM") as psp,
        ):
            zero = constp.tile([P, 1], f32)
            nc.any.memset(zero[:], 0.0)

            # HAM warm-up: ~3.5us of dependency-free matmuls on a memset tile
            # fill the PE activity window while xe/w1 stream in, so the real
            # matmuls start at 2.4GHz instead of the cold 1.2GHz
            warm_w = constp.tile([P, P], bf16)
            nc.any.memset(warm_w[:], 0.0)
            warm_sb = constp.tile([P, 512], bf16)
            nc.any.memset(warm_sb[:], 0.0)
            warm_out = constp.tile([P, 1], f32)
            with tc.tile_pool(name="warmps", bufs=1, space="PSUM") as warmp:
                warm_ps = warmp.tile([P, 512], f32)
                N_WARM = 8
                for i in range(N_WARM):
                    nc.tensor.matmul(
                        warm_ps[:], warm_w[:], warm_sb[:], start=True, stop=True
                    )
                nc.vector.tensor_copy(warm_out[:], warm_ps[:, :1])

            # first stationary tile ahead of everything: it heads its DMA
            # queue so the PE can start early
            w1_first = w1p.tile([P, KO, P], bf16, name="w1_t")
            nc.sync.dma_start(w1_first[:], w1[0])

            # xe per ko in consumption order so MM1(fc0) streams as they land
            xe_sb = xep.tile([P, KO, c_act], bf16)
            for ko in range(KO):
                nc.sync.dma_start(xe_sb[:, ko, :], xeT_r[:, ko, :])

            hT = htp.tile([P, NF, c_act], bf16)
            w2_sb = w2p.tile([P, NF, D], bf16)
            yT_sb = yp.tile([P, NDC, c_act], bf16)

            # chunk index -> (c offset, width)
            offs = []
            c0 = 0
            for cw in chunks:
                offs.append((c0, cw))
                c0 += cw
            # smallest chunk first: the trailing wide matmul hides the next
            # k-step's LDWEIGHTS
            order = sorted(range(len(chunks)), key=lambda j: chunks[j])

            # ---- MM1: hT = relu(W1.T @ xeT), f-chunk at a time ----
            for fc in range(NF):
                if fc == 0:
                    w1_t = w1_first
                else:
                    w1_t = w1p.tile([P, KO, P], bf16, name="w1_t")
                    nc.sync.dma_start(w1_t[:], w1[fc])
                # pace the W2 prefetch behind the w1 tiles in queue order;
                # all of W2 is resident in SBUF before MM2 begins
                nc.sync.dma_start(w2_sb[:, fc, :], w2[fc * P : (fc + 1) * P, :])
                ph = {
                    i: psp.tile([P, chunks[i]], f32, name=f"p{i}", tag=f"p{i}")
                    for i in range(len(chunks))
                }
                for k in range(KO):
                    for i in order:
                        c0, cw = offs[i]
                        nc.tensor.matmul(
                            ph[i][:],
                            w1_t[:, k, :],
                            xe_sb[:, k, c0 : c0 + cw],
                            start=(k == 0),
                            stop=(k == KO - 1),
                        )
                for i in range(len(chunks)):
                    c0, cw = offs[i]
                    nc.scalar.activation(
                        hT[:, fc, c0 : c0 + cw], ph[i][:], Relu, bias=zero[:]
                    )

            # ---- MM2: yT = W2.T @ hT, single full-F sweep per d-chunk ----
            # stationary = W2 [128f, 128d] chunks, moving = hT c-chunks
            # (smallest chunk first so the trailing wide matmul hides the
            # next f-step's LDWEIGHTS). Casts alternate Vector/Scalar
            # engines; stores stream per chunk so only the last ~1.5us of
            # store work is exposed.
            for dh in range(NDC):
                py = {
                    i: psp.tile([P, chunks[i]], f32, name=f"py{i}", tag=f"p{i}")
                    for i in range(len(chunks))
                }
                for fs in range(NF):
                    for i in order:
                        c0, cw = offs[i]
                        nc.tensor.matmul(
                            py[i][:],
                            w2_sb[:, fs, dh * P : (dh + 1) * P],
                            hT[:, fs, c0 : c0 + cw],
                            start=(fs == 0),
                            stop=(fs == NF - 1),
                        )
                for j, i in enumerate(order):
                    c0, cw = offs[i]
                    dst = yT_sb[:, dh, c0 : c0 + cw]
                    if j % 2 == 0:
                        nc.vector.tensor_copy(dst, py[i][:])
                    else:
                        nc.scalar.activation(dst, py[i][:], Ident, bias=zero[:])
                    nc.sync.dma_start(
                        yT[dh * P : (dh + 1) * P, c0 : c0 + cw], dst
                    )

    nc.compile()
    names = dict(w1=w1.name, w2=w2.name, xeT=xeT.name, y=yT.name)
    return nc, names


def _get_program(c_act):
    if c_act not in _PROGRAMS:
        _PROGRAMS[c_act] = _build_program(c_act)
    return _PROGRAMS[c_act]


# test.py can set RUN_KWARGS (e.g. dict(trace=True)) and read LAST_RESULTS
RUN_KWARGS = {}
LAST_RESULTS = None


def kernel(x, route_mask, route_weight, W1, b1, W2, b2):
    import ml_dtypes

    from concourse.bass_utils import run_bass_kernel_spmd

    global LAST_RESULTS

    bf = ml_dtypes.bfloat16

    x = np.asarray(x, dtype=np.float32)
    route_mask = np.asarray(route_mask, dtype=bool)
    route_weight = np.asarray(route_weight, dtype=np.float32)
    W1 = np.asarray(W1, dtype=np.float32)
    W2 = np.asarray(W2, dtype=np.float32)
    b1 = np.asarray(b1, dtype=np.float32)
    b2 = np.asarray(b2, dtype=np.float32)
    if np.any(b1):
        raise NotImplementedError("nonzero b1 not supported")

    # --- routing: per-expert top-C tokens by route weight (ties -> lower idx) ---
    w_et = np.where(route_mask.T, route_weight.T, -np.inf)  # (E, T)
    order = np.argsort(-w_et, axis=1, kind="stable")[:, :C]  # (E, C) token ids
    vals = np.take_along_axis(w_et, order, axis=1)  # (E, C)
    valid = np.isfinite(vals)  # (E, C)
    gain = np.where(valid, vals, 0.0).astype(np.float32)  # (E, C)

    # active capacity: valid slots are a prefix (sorted by weight desc)
    n_e = valid.sum(axis=1)
    c_act = min(C, int(math.ceil(max(1, n_e.max()) / 16)) * 16)

    nc, names = _get_program(c_act)

    # --- dispatch: gather + fold gain, per expert ---
    in_maps = []
    for e in range(E):
        xe = x[order[e, :c_act]] * gain[e, :c_act][:, None]  # (Ca, D)
        xeT_np = np.ascontiguousarray(xe.T.astype(bf))  # (D, Ca)
        w1b = np.ascontiguousarray(
            W1[e].reshape(KO, P, NF, P).transpose(2, 1, 0, 3).astype(bf)
        )  # (NF, P, KO, P)
        in_maps.append(
            {names["w1"]: w1b, names["xeT"]: xeT_np, names["w2"]: W2[e].astype(bf)}
        )

    res = run_bass_kernel_spmd(nc, in_maps, list(range(N_CORES)), **RUN_KWARGS)
    LAST_RESULTS = res

    # --- combine: transpose + scatter-add per-expert outputs ---
    y = np.zeros((T, D), np.float32)
    for e in range(E):
        ye = res.results[e][names["y"]].T.astype(np.float32)  # (Ca, D)
        m = valid[e, :c_act]
        if np.any(b2):
            ye = ye + gain[e, :c_act][:, None] * b2[e][None, :]
        y[order[e, :c_act][m]] += ye[m]
    return y


# revision 11
# speedup vs baseline: 1.1694x; 1.0601x over previous
"""MoE Trainium2 kernel v4: two-segment SPMD load balancing, all-bf16.

Every core runs the SAME program shape: two token segments of sizes (a, b),
each segment paired with its own W1/W2 weight inputs. Heavy experts (whose
routed count exceeds the segment budget) are split across two cores'
a-segments; light experts pair up in b-segments. The (a, b) sizes are chosen
per-input by a tiny solver (k experts split a+a, 8-2k run a+b, k pair b+b)
minimizing a+b — for balanced routing this lands ~4% above the perfect
sum/8 split vs ~11% for one-expert-per-core.

Device math per core (both segments, bf16):
    hT = relu(W1_s.T @ xeT_s)   (F, a|b)  in one SBUF tile [128, NF, a+b]
    yT = W2_s.T @ hT_s          (D, a|b)  W2 128x128 chunks stationary

W2 is host-packed per d-chunk (NDC, P, NF, 128) and streamed during MM2;
W1 is host-packed (NF, P, KO, P) and streamed during MM1.
"""

import math
import sys

import numpy as np

for _p in ("/opt/trn_rl_repo",):
    if _p not in sys.path:
        sys.path.append(_p)

T, E, D, F, C, K = 4096, 8, 1024, 4096, 1536, 2
N_CORES = 8
P = 128
KO = D // P
NF = F // P
NDC = D // P

_PROGRAMS = {}


def _plan_segments(n_e):
    """Pick segment sizes (a, b) and assign experts to the 8 a-segs + 8 b-segs.

    Returns (a, b, a_segs, b_segs) where each seg list has 8 entries
    (expert, src_off, cnt): the segment holds slots [src_off, src_off+cnt)
    of that expert's gain-sorted slot list (cnt may be 0 for pad segments).
    """
    ns = sorted(range(E), key=lambda e: -n_e[e])  # experts by load desc
    best = None
    for k in range(0, E // 2 + 1):
        top = [n_e[e] for e in ns[:k]]
        mid = [n_e[e] for e in ns[k : E - k]]
        bot = [n_e[e] for e in ns[E - k :]]
        a_min = max([(v + 1) // 2 for v in top], default=0)
        b_min = max([(v + 1) // 2 for v in bot], default=0)
        mid_max = max(mid, default=0)
        a = max(a_min, (mid_max + 1) // 2, 16)
        b = max(b_min, mid_max - a, 16)
        a = (a + 7) // 8 * 8
        b = (b + 7) // 8 * 8
        if best is None or a + b < best[0] + best[1]:
            best = (a, b, k)
    a, b, k = best
    a_segs, b_segs = [], []
    for i, e in enumerate(ns):
        n = n_e[e]
        if i < k:  # a + a
            a_segs.append((e, 0, min(a, n)))
            a_segs.append((e, min(a, n), max(0, n - a)))
        elif i < E - k:  # a + b
            a_segs.append((e, 0, min(a, n)))
            b_segs.append((e, min(a, n), max(0, n - a)))
        else:  # b + b
            b_segs.append((e, 0, min(b, n)))
            b_segs.append((e, min(b, n), max(0, n - b)))
    assert len(a_segs) == N_CORES and len(b_segs) == N_CORES
    return a, b, a_segs, b_segs


def _seg_chunks(width, base):
    """<=512-wide chunk list for one segment, smallest chunk first."""
    out = []
    rem, c0 = width, base
    while rem > 0:
        take = min(512, rem)
        out.append((c0, take))
        c0 += take
        rem -= take
    out.sort(key=lambda t: t[1])
    return out


def _build_program(a, b):
    import concourse.mybir as mybir
    import concourse.tile as tile
    from concourse import bacc

    f32 = mybir.dt.float32
    bf16 = mybir.dt.bfloat16
    Relu = mybir.ActivationFunctionType.Relu
    Ident = mybir.ActivationFunctionType.Identity

    B = a + b
    # per-segment chunk lists; global tag numbering across both
    seg_chunks = [_seg_chunks(a, 0), _seg_chunks(b, a)]
    flat = [(s, c0, cw) for s in (0, 1) for (c0, cw) in seg_chunks[s]]

    nc = bacc.Bacc(None, target_bir_lowering=False, debug=False)

    with tile.TileContext(nc) as tc:
        with tc.tile_pool(name="dram", bufs=1, space="DRAM") as dram:
            w1s = [
                dram.tile((NF, P, KO, P), bf16, kind="ExternalInput", name=f"w1{s}")
                for s in range(2)
            ]
            # w2 packed per d-chunk: (NDC, P, NF, 128)
            w2s = [
                dram.tile((NDC, P, NF, P), bf16, kind="ExternalInput", name=f"w2{s}")
                for s in range(2)
            ]
            xeT = dram.tile((D, B), bf16, kind="ExternalInput", name="xeT")
            yT = dram.tile((D, B), bf16, kind="ExternalOutput", name="yT")

        xeT_r = xeT[:].rearrange("(ko ki) c -> ki ko c", ki=P)

        with (
            tc.tile_pool(name="const", bufs=1) as constp,
            tc.tile_pool(name="xe", bufs=1) as xep,
            tc.tile_pool(name="ht", bufs=1) as htp,
            tc.tile_pool(name="ysb", bufs=1) as yp,
            tc.tile_pool(name="w1t", bufs=8) as w1p,
            tc.tile_pool(name="w2t", bufs=4) as w2p,
            tc.tile_pool(name="ps", bufs=2, space="PSUM") as psp,
        ):
            zero = constp.tile([P, 1], f32)
            nc.any.memset(zero[:], 0.0)

            warm_w = constp.tile([P, P], bf16)
            nc.any.memset(warm_w[:], 0.0)
            warm_sb = constp.tile([P, 512], bf16)
            nc.any.memset(warm_sb[:], 0.0)
            warm_out = constp.tile([P, 1], f32)
            with tc.tile_pool(name="warmps", bufs=1, space="PSUM") as warmp:
                warm_ps = warmp.tile([P, 512], f32)
                for i in range(8):
                    nc.tensor.matmul(
                        warm_ps[:], warm_w[:], warm_sb[:], start=True, stop=True
                    )
                nc.vector.tensor_copy(warm_out[:], warm_ps[:, :1])

            # first stationary tiles head their DMA queues
            w1_first = [w1p.tile([P, KO, P], bf16, name="w1_t") for s in range(2)]
            for s in range(2):
                nc.sync.dma_start(w1_first[s][:], w1s[s][0])

            xe_sb = xep.tile([P, KO, B], bf16)
            for ko in range(KO):
                nc.sync.dma_start(xe_sb[:, ko, :], xeT_r[:, ko, :])

            hT = htp.tile([P, NF, B], bf16)
            yT_sb = yp.tile([P, NDC, B], bf16)

            # W2 d-chunk tiles are streamed; first two d-chunks prefetch
            # during the tail of MM1 (after the w1 tiles they'd contend with)
            w2_t = {}

            # ---- MM1 ----
            for fc in range(NF):
                w1_t = []
                for s in range(2):
                    if fc == 0:
                        w1_t.append(w1_first[s])
                    else:
                        t = w1p.tile([P, KO, P], bf16, name="w1_t")
                        nc.sync.dma_start(t[:], w1s[s][fc])
                        w1_t.append(t)
                if fc >= NF - 4:
                    dh, s = divmod(fc - (NF - 4), 2)
                    w2_t[(dh, s)] = w2p.tile([P, NF, P], bf16, name="w2_t")
                    nc.sync.dma_start(w2_t[(dh, s)][:], w2s[s][dh])
                ph = {
                    i: psp.tile([P, cw], f32, name=f"p{i}", tag=f"p{i}")
                    for i, (s, c0, cw) in enumerate(flat)
                }
                for k in range(KO):
                    for i, (s, c0, cw) in enumerate(flat):
                        nc.tensor.matmul(
                            ph[i][:],
                            w1_t[s][:, k, :],
                            xe_sb[:, k, c0 : c0 + cw],
                            start=(k == 0),
                            stop=(k == KO - 1),
                        )
                for i, (s, c0, cw) in enumerate(flat):
                    nc.scalar.activation(
                        hT[:, fc, c0 : c0 + cw], ph[i][:], Relu, bias=zero[:]
                    )

            # ---- MM2 ----
            for dh in range(NDC):
                if dh + 2 < NDC:
                    for s in range(2):
                        w2_t[(dh + 2, s)] = w2p.tile([P, NF, P], bf16, name="w2_t")
                        nc.sync.dma_start(w2_t[(dh + 2, s)][:], w2s[s][dh + 2])
                py = {
                    i: psp.tile([P, cw], f32, name=f"py{i}", tag=f"p{i}")
                    for i, (s, c0, cw) in enumerate(flat)
                }
                for fs in range(NF):
                    for i, (s, c0, cw) in enumerate(flat):
                        nc.tensor.matmul(
                            py[i][:],
                            w2_t[(dh, s)][:, fs, :],
                            hT[:, fs, c0 : c0 + cw],
                            start=(fs == 0),
                            stop=(fs == NF - 1),
                        )
                for i, (s, c0, cw) in enumerate(flat):
                    dst = yT_sb[:, dh, c0 : c0 + cw]
                    if i % 2 == 0:
                        nc.vector.tensor_copy(dst, py[i][:])
                    else:
                        nc.scalar.activation(dst, py[i][:], Ident, bias=zero[:])
                    nc.sync.dma_start(yT[dh * P : (dh + 1) * P, c0 : c0 + cw], dst)

    nc.compile()
    names = dict(
        w1=[t.name for t in w1s],
        w2=[t.name for t in w2s],
        xeT=xeT.name,
        y=yT.name,
    )
    return nc, names


def _get_program(a, b):
    if (a, b) not in _PROGRAMS:
        _PROGRAMS[(a, b)] = _build_program(a, b)
    return _PROGRAMS[(a, b)]


RUN_KWARGS = {}
LAST_RESULTS = None


def kernel(x, route_mask, route_weight, W1, b1, W2, b2):
    import ml_dtypes

    from concourse.bass_utils import run_bass_kernel_spmd

    global LAST_RESULTS

    bf = ml_dtypes.bfloat16

    x = np.asarray(x, dtype=np.float32)
    route_mask = np.asarray(route_mask, dtype=bool)
    route_weight = np.asarray(route_weight, dtype=np.float32)
    W1 = np.asarray(W1, dtype=np.float32)
    W2 = np.asarray(W2, dtype=np.float32)
    b1 = np.asarray(b1, dtype=np.float32)
    b2 = np.asarray(b2, dtype=np.float32)
    if np.any(b1):
        raise NotImplementedError("nonzero b1 not supported")

    w_et = np.where(route_mask.T, route_weight.T, -np.inf)  # (E, T)
    order = np.argsort(-w_et, axis=1, kind="stable")[:, :C]  # (E, C)
    vals = np.take_along_axis(w_et, order, axis=1)
    valid = np.isfinite(vals)
    gain = np.where(valid, vals, 0.0).astype(np.float32)

    n_e = np.minimum(valid.sum(axis=1), C).astype(int)
    a, b, a_segs, b_segs = _plan_segments(n_e)
    B = a + b

    nc, names = _get_program(a, b)

    # pre-pack per-expert weights once (an expert may appear on 2 cores)
    used = sorted({e for e, _, cnt in a_segs + b_segs if cnt > 0})
    w1p_, w2p_ = {}, {}
    for e in used:
        w1p_[e] = np.ascontiguousarray(
            W1[e].reshape(KO, P, NF, P).transpose(2, 1, 0, 3).astype(bf)
        )
        w2p_[e] = np.ascontiguousarray(
            W2[e].reshape(NF, P, NDC, P).transpose(2, 1, 0, 3).astype(bf)
        )
    w1_pad = np.zeros((NF, P, KO, P), bf)
    w2_pad = np.zeros((NDC, P, NF, P), bf)

    in_maps = []
    for core in range(N_CORES):
        segs = [(a_segs[core], 0, a), (b_segs[core], a, b)]
        xeT_np = np.zeros((D, B), bf)
        im = {}
        for s, ((e, off, cnt), base, width) in enumerate(segs):
            if cnt > 0:
                idx = order[e, off : off + cnt]
                xe = x[idx] * gain[e, off : off + cnt][:, None]
                xeT_np[:, base : base + cnt] = xe.T.astype(bf)
                im[names["w1"][s]] = w1p_[e]
                im[names["w2"][s]] = w2p_[e]
            else:
                im[names["w1"][s]] = w1_pad
                im[names["w2"][s]] = w2_pad
        im[names["xeT"]] = np.ascontiguousarray(xeT_np)
        in_maps.append(im)

    res = run_bass_kernel_spmd(nc, in_maps, list(range(N_CORES)), **RUN_KWARGS)
    LAST_RESULTS = res

    y = np.zeros((T, D), np.float32)
    for core in range(N_CORES):
        yTc = res.results[core][names["y"]]
        for (e, off, cnt), base, width in (
            (a_segs[core], 0, a),
            (b_segs[core], a, b),
        ):
            if cnt == 0:
                continue
            ye = yTc[:, base : base + cnt].T.astype(np.float32)
            if np.any(b2):
                ye = ye + gain[e, off : off + cnt][:, None] * b2[e][None, :]
            y[order[e, off : off + cnt]] += ye
    return y
